# revision 22
# baseline (speedup 1.0000x reference)
"""Trainium2 Bass kernel for nn_CCM: per-pixel complex 3x3 mask stencil.

Computation (per batch b):
  H_c = m[c] + v1*m[9+c] + v2*m[18+c],  v1/v2 = -1/2 +- i*sqrt(3)/2, c in 0..8
  out(t,f) = sum_c H_c(t,f) * xpad(t + c//3, f + c%3)   (complex)
with xpad zero-padded by 2 rows at the top (causal time) and 1 col each side.

Sharding: pure data-parallel over B=8 across the 8 NeuronCores.

v4.5 design (informed by per-run traces):
  - Host precomputes mask channels hre_c, him_c (fp32 -> bf16); the
    Karatsuba sum channel hsum=hre+him is derived on-chip (1 DVE add
    per tap).  Host also ships s=xr+xi planes.
  - One stacked DVE product op per tap over the (k1|k2|k3) plane axis;
    in-place balanced-tree accumulation; bf16 2x mode throughout.
    GpSimd does no elementwise work (SBUF-port contention).
  - DMA split by measured path capability:
      SWDGE (gpsimd, 16 engines, ~14-18GB/s each): 9 single-tap mask
        DMAs (ring of 4 for lookahead) + 6 s-plane chunks.
      HWDGE (sync/scalar, 5 shared engines, ~26GB/s each): x planes
        (xr, xi) both parities, plus the two output stores.
    Every transfer is >=8KB contiguous per partition.
  - x planes replicated for the 3 row shifts and duplicated at two
    byte parities so all product slices stay 4B-aligned (2x mode).
  - Error ~1.3e-2 scale-relative (budget 2e-2).
"""

import sys

import numpy as np

sys.path.insert(0, "/opt/trn_rl_repo")

B, T, F = 8, 1000, 257
TP = 125          # partitions; time row t = kk*TP + p
KK = 8            # time chunks
FB = 258          # padded op width (even element count for bf16 2x mode)
XE = 260          # even-parity x row width (covers col shifts 0 and 2)
XO = 258          # odd-parity x row width (col shift 1, pre-shifted)
SQ3_2 = float(np.sqrt(3.0) / 2.0)

# tap compute order; masks are stored in this order tap-major
TAP_ORDER = (0, 2, 1, 3, 5, 4, 6, 8, 7)

_prog_cache = {}


def _build_program():
    import concourse.tile as tile
    from concourse import bacc, mybir

    bf16 = mybir.dt.bfloat16

    nc = bacc.Bacc()
    # (hre, him) per tap, partition-major, tap axis in TAP_ORDER
    mk_d = nc.declare_dram_parameter("mk", [TP, 9, 2, KK, FB], bf16,
                                     isOutput=False)
    # x planes (xr, xi) per (rowshift rep, parity)
    xe_d = nc.declare_dram_parameter("xe", [3, TP, 2, KK, XE], bf16,
                                     isOutput=False)
    xo_d = nc.declare_dram_parameter("xo", [3, TP, 2, KK, XO], bf16,
                                     isOutput=False)
    # s = xr + xi planes per (rep, parity)
    se_d = nc.declare_dram_parameter("se", [3, TP, KK, XE], bf16,
                                     isOutput=False)
    so_d = nc.declare_dram_parameter("so", [3, TP, KK, XO], bf16,
                                     isOutput=False)
    ore_d = nc.declare_dram_parameter("ore", [TP, KK, FB], bf16,
                                      isOutput=True)
    oim_d = nc.declare_dram_parameter("oim", [TP, KK, FB], bf16,
                                      isOutput=True)

    with tile.TileContext(nc) as tc:
        from contextlib import ExitStack

        with ExitStack() as ctx:
            xpool = ctx.enter_context(tc.tile_pool(name="xpool", bufs=1))
            mpool = ctx.enter_context(tc.tile_pool(name="mpool", bufs=4))
            ppool = ctx.enter_context(tc.tile_pool(name="ppool", bufs=1))
            opool = ctx.enter_context(tc.tile_pool(name="opool", bufs=1))

            xe_t = {}
            xo_t = {}
            mk_t = {}

            def load_x(rep):
                te = xpool.tile([TP, 3, KK, XE], bf16, tag=f"xe{rep}",
                                name=f"xe{rep}")
                nc.sync.dma_start(out=te[:, 0:2], in_=xe_d[rep])
                nc.gpsimd.dma_start(out=te[:, 2], in_=se_d[rep])
                xe_t[rep] = te
                to = xpool.tile([TP, 3, KK, XO], bf16, tag=f"xo{rep}",
                                name=f"xo{rep}")
                nc.scalar.dma_start(out=to[:, 0:2], in_=xo_d[rep])
                nc.gpsimd.dma_start(out=to[:, 2], in_=so_d[rep])
                xo_t[rep] = to

            def load_mk(oi):
                t = mpool.tile([TP, 3, KK, FB], bf16, tag="mk",
                               name=f"mk{oi}")
                nc.gpsimd.dma_start(out=t[:, 0:2], in_=mk_d[:, oi])
                mk_t[TAP_ORDER[oi]] = t

            # SWDGE stream: first tap's masks + s first; HWDGE carries
            # the xr/xi planes in parallel on its own queues.
            load_mk(0)
            load_x(0)
            load_mk(1)
            load_mk(2)
            load_x(1)
            load_mk(3)
            load_mk(4)
            load_x(2)
            load_mk(5)
            load_mk(6)
            load_mk(7)
            load_mk(8)

            # ---- Compute: hsum derivation + one stacked product per
            # tap; balanced pairwise tree with in-place accumulation:
            #   (((0+2)+(1+3)) + ((5+4)+(6+8))) + 7
            def prod_op(c, tag, bufs=1):
                mm, nn = divmod(c, 3)
                if nn == 1:
                    xs = xo_t[mm][:, :, :, 0:FB]
                else:
                    xs = xe_t[mm][:, :, :, nn:nn + FB]
                mk = mk_t[c]
                nc.vector.tensor_add(mk[:, 2], mk[:, 0], mk[:, 1])
                p = ppool.tile([TP, 3, KK, FB], bf16, tag=tag, bufs=bufs,
                               name=f"p{c}")
                nc.vector.tensor_mul(p, mk, xs)
                return p

            acc0 = prod_op(0, "acc0")                 # p0
            f0 = prod_op(2, "feed", bufs=2)
            nc.vector.tensor_add(acc0, acc0, f0)      # 0+2
            t0 = prod_op(1, "t0")
            f1 = prod_op(3, "feed", bufs=2)
            nc.vector.tensor_add(t0, t0, f1)          # 1+3
            nc.vector.tensor_add(acc0, acc0, t0)      # (0+2)+(1+3)
            acc1 = prod_op(5, "acc1")
            f2 = prod_op(4, "feed", bufs=2)
            nc.vector.tensor_add(acc1, acc1, f2)      # 5+4
            t1 = prod_op(6, "t1")
            f3 = prod_op(8, "feed", bufs=2)
            nc.vector.tensor_add(t1, t1, f3)          # 6+8
            nc.vector.tensor_add(acc1, acc1, t1)      # (5+4)+(6+8)
            nc.vector.tensor_add(acc0, acc0, acc1)    # left + right
            f4 = prod_op(7, "feed", bufs=2)
            nc.vector.tensor_add(acc0, acc0, f4)      # ... + 7
            A = acc0[:, 0]
            Bc = acc0[:, 1]
            Cc = acc0[:, 2]

            # merges; store re and im on separate HWDGE queues so the
            # stores overlap the remaining compute
            out_re = opool.tile([TP, KK, FB], bf16, tag="ore")
            nc.vector.tensor_sub(out_re, A, Bc)                # re = A - B
            nc.sync.dma_start(out=ore_d[:, :, :], in_=out_re)
            tsum = ppool.tile([TP, 1, KK, FB], bf16, tag="t0", name="tsum")
            nc.vector.tensor_add(tsum[:, 0], A, Bc)
            out_im = opool.tile([TP, KK, FB], bf16, tag="oim")
            nc.vector.tensor_sub(out_im, Cc, tsum[:, 0])       # im = C-(A+B)
            nc.scalar.dma_start(out=oim_d[:, :, :], in_=out_im)

    nc.finalize()
    return nc


def _get_program():
    if "nc" not in _prog_cache:
        _prog_cache["nc"] = _build_program()
    return _prog_cache["nc"]


def _host_prep(m, x):
    import ml_dtypes

    bf = ml_dtypes.bfloat16
    in_maps = []
    for b in range(B):
        mr = m[b].reshape(3, 9, T, F)
        hre = mr[0] - 0.5 * (mr[1] + mr[2])
        him = SQ3_2 * (mr[1] - mr[2])
        # [2ch, 9, T, F] -> [TP, 9(tap order), 2, KK, FB]; t = kk*TP + p
        mk = np.zeros((2, 9, KK, TP, FB), np.float32)
        st = np.stack([hre, him])[:, list(TAP_ORDER)]  # (2, 9, T, F)
        mk[:, :, :, :, :F] = st.reshape(2, 9, KK, TP, F)
        mk = np.ascontiguousarray(mk.transpose(3, 1, 0, 2, 4)).astype(bf)

        xb = x[b]  # (F, T, 2)
        xrp = np.zeros((T + 2, XE + 2), np.float32)
        xip = np.zeros((T + 2, XE + 2), np.float32)
        xrp[2:, 1:F + 1] = xb[:, :, 0].T
        xip[2:, 1:F + 1] = xb[:, :, 1].T
        sp = xrp + xip
        planes = [xrp, xip]
        xe = np.empty((3, TP, 2, KK, XE), np.float32)
        xo = np.empty((3, TP, 2, KK, XO), np.float32)
        se = np.empty((3, TP, KK, XE), np.float32)
        so = np.empty((3, TP, KK, XO), np.float32)
        for rep in range(3):
            for kk in range(KK):
                # rows t = kk*TP + p, padded row index t + rep
                r0 = kk * TP + rep
                for pl in range(2):
                    xe[rep, :, pl, kk, :] = planes[pl][r0:r0 + TP, 0:XE]
                    xo[rep, :, pl, kk, :] = planes[pl][r0:r0 + TP, 1:1 + XO]
                se[rep, :, kk, :] = sp[r0:r0 + TP, 0:XE]
                so[rep, :, kk, :] = sp[r0:r0 + TP, 1:1 + XO]
        in_maps.append({"mk": mk, "xe": xe.astype(bf), "xo": xo.astype(bf),
                        "se": se.astype(bf), "so": so.astype(bf)})
    return in_maps


def _assemble(results):
    out = np.empty((B, F, T, 2), np.float32)
    for b in range(B):
        for ci, name in enumerate(("ore", "oim")):
            arr = results[b][name].astype(np.float32)  # [TP, KK, FB]
            a = arr[:, :, :F].transpose(1, 0, 2).reshape(T, F)  # t = kk*TP+p
            out[b, :, :, ci] = a.T
    return out


def kernel(m, x, _trace=False):
    from concourse.bass_utils import run_bass_kernel_spmd

    nc = _get_program()
    in_maps = _host_prep(np.asarray(m), np.asarray(x))
    res = run_bass_kernel_spmd(nc, in_maps, list(range(B)), trace=_trace)
    out = _assemble(res.results)
    if _trace:
        return out, res
    return out


# revision 23
# speedup vs baseline: 1.1344x; 1.1344x over previous
"""Trainium2 Bass kernel for nn_CCM: per-pixel complex 3x3 mask stencil.

Computation (per batch b):
  H_c = m[c] + v1*m[9+c] + v2*m[18+c],  v1/v2 = -1/2 +- i*sqrt(3)/2, c in 0..8
  out(t,f) = sum_c H_c(t,f) * xpad(t + c//3, f + c%3)   (complex)
with xpad zero-padded by 2 rows at the top (causal time) and 1 col each side.

Sharding: pure data-parallel over B=8 across the 8 NeuronCores.

v4.5 design (informed by per-run traces):
  - Host precomputes mask channels hre_c, him_c (fp32 -> bf16); the
    Karatsuba sum channel hsum=hre+him is derived on-chip (1 DVE add
    per tap).  Host also ships s=xr+xi planes.
  - One stacked DVE product op per tap over the (k1|k2|k3) plane axis;
    in-place balanced-tree accumulation; bf16 2x mode throughout.
    GpSimd does no elementwise work (SBUF-port contention).
  - DMA split by measured path capability:
      SWDGE (gpsimd, 16 engines, ~14-18GB/s each): 9 single-tap mask
        DMAs (ring of 4 for lookahead) + 6 s-plane chunks.
      HWDGE (sync/scalar, 5 shared engines, ~26GB/s each): x planes
        (xr, xi) both parities, plus the two output stores.
    Every transfer is >=8KB contiguous per partition.
  - x planes replicated for the 3 row shifts and duplicated at two
    byte parities so all product slices stay 4B-aligned (2x mode).
  - Error ~1.3e-2 scale-relative (budget 2e-2).
"""

import sys

import numpy as np

sys.path.insert(0, "/opt/trn_rl_repo")

B, T, F = 8, 1000, 257
TP = 125          # partitions; time row t = kk*TP + p
KK = 8            # time chunks
FB = 258          # padded op width (even element count for bf16 2x mode)
XE = 260          # even-parity x row width (covers col shifts 0 and 2)
XO = 258          # odd-parity x row width (col shift 1, pre-shifted)
SQ3_2 = float(np.sqrt(3.0) / 2.0)

# tap compute order; masks are stored in this order tap-major
TAP_ORDER = (0, 2, 1, 3, 5, 4, 6, 8, 7)

_prog_cache = {}


def _build_program():
    import concourse.tile as tile
    from concourse import bacc, mybir

    bf16 = mybir.dt.bfloat16

    nc = bacc.Bacc()
    # (hre, him) per tap, partition-major, tap axis in TAP_ORDER
    mk_d = nc.declare_dram_parameter("mk", [TP, 9, 2, KK, FB], bf16,
                                     isOutput=False)
    # x planes (xr, xi) per (rowshift rep, parity)
    xe_d = nc.declare_dram_parameter("xe", [3, TP, 2, KK, XE], bf16,
                                     isOutput=False)
    xo_d = nc.declare_dram_parameter("xo", [3, TP, 2, KK, XO], bf16,
                                     isOutput=False)
    # s = xr + xi planes per (rep, parity)
    se_d = nc.declare_dram_parameter("se", [3, TP, KK, XE], bf16,
                                     isOutput=False)
    so_d = nc.declare_dram_parameter("so", [3, TP, KK, XO], bf16,
                                     isOutput=False)
    ore_d = nc.declare_dram_parameter("ore", [TP, KK, FB], bf16,
                                      isOutput=True)
    oim_d = nc.declare_dram_parameter("oim", [TP, KK, FB], bf16,
                                      isOutput=True)

    with tile.TileContext(nc) as tc:
        from contextlib import ExitStack

        with ExitStack() as ctx:
            xpool = ctx.enter_context(tc.tile_pool(name="xpool", bufs=1))
            mpool = ctx.enter_context(tc.tile_pool(name="mpool", bufs=4))
            ppool = ctx.enter_context(tc.tile_pool(name="ppool", bufs=1))
            opool = ctx.enter_context(tc.tile_pool(name="opool", bufs=1))

            xe_t = {}
            xo_t = {}
            mk_t = {}

            # Everything rides ONE SWDGE queue: concurrent HWDGE
            # traffic oversubscribes the 5 shared SDMA engines and
            # delays the completion of EVERY SWDGE DMA (each needs all
            # 16 engines to finish).  HWDGE is used only for the final
            # output stores, after the SWDGE stream has drained.
            def load_x(rep):
                te = xpool.tile([TP, 3, KK, XE], bf16, tag=f"xe{rep}",
                                name=f"xe{rep}")
                nc.gpsimd.dma_start(out=te[:, 0:2], in_=xe_d[rep])
                nc.gpsimd.dma_start(out=te[:, 2], in_=se_d[rep])
                xe_t[rep] = te
                to = xpool.tile([TP, 3, KK, XO], bf16, tag=f"xo{rep}",
                                name=f"xo{rep}")
                nc.gpsimd.dma_start(out=to[:, 0:2], in_=xo_d[rep])
                nc.gpsimd.dma_start(out=to[:, 2], in_=so_d[rep])
                xo_t[rep] = to

            def load_mk(oi):
                t = mpool.tile([TP, 3, KK, FB], bf16, tag="mk",
                               name=f"mk{oi}")
                nc.gpsimd.dma_start(out=t[:, 0:2], in_=mk_d[:, oi])
                mk_t[TAP_ORDER[oi]] = t

            load_mk(0)
            load_x(0)
            load_mk(1)
            load_mk(2)
            load_x(1)
            load_mk(3)
            load_mk(4)
            load_x(2)
            load_mk(5)
            load_mk(6)
            load_mk(7)
            load_mk(8)

            # ---- Compute: hsum derivation + one stacked product per
            # tap; balanced pairwise tree with in-place accumulation:
            #   (((0+2)+(1+3)) + ((5+4)+(6+8))) + 7
            def prod_op(c, tag, bufs=1):
                mm, nn = divmod(c, 3)
                if nn == 1:
                    xs = xo_t[mm][:, :, :, 0:FB]
                else:
                    xs = xe_t[mm][:, :, :, nn:nn + FB]
                mk = mk_t[c]
                nc.vector.tensor_add(mk[:, 2], mk[:, 0], mk[:, 1])
                p = ppool.tile([TP, 3, KK, FB], bf16, tag=tag, bufs=bufs,
                               name=f"p{c}")
                nc.vector.tensor_mul(p, mk, xs)
                return p

            acc0 = prod_op(0, "acc0")                 # p0
            f0 = prod_op(2, "feed", bufs=2)
            nc.vector.tensor_add(acc0, acc0, f0)      # 0+2
            t0 = prod_op(1, "t0")
            f1 = prod_op(3, "feed", bufs=2)
            nc.vector.tensor_add(t0, t0, f1)          # 1+3
            nc.vector.tensor_add(acc0, acc0, t0)      # (0+2)+(1+3)
            acc1 = prod_op(5, "acc1")
            f2 = prod_op(4, "feed", bufs=2)
            nc.vector.tensor_add(acc1, acc1, f2)      # 5+4
            t1 = prod_op(6, "t1")
            f3 = prod_op(8, "feed", bufs=2)
            nc.vector.tensor_add(t1, t1, f3)          # 6+8
            nc.vector.tensor_add(acc1, acc1, t1)      # (5+4)+(6+8)
            nc.vector.tensor_add(acc0, acc0, acc1)    # left + right
            f4 = prod_op(7, "feed", bufs=2)
            nc.vector.tensor_add(acc0, acc0, f4)      # ... + 7
            A = acc0[:, 0]
            Bc = acc0[:, 1]
            Cc = acc0[:, 2]

            # merges; store re and im on separate HWDGE queues so the
            # stores overlap the remaining compute
            out_re = opool.tile([TP, KK, FB], bf16, tag="ore")
            nc.vector.tensor_sub(out_re, A, Bc)                # re = A - B
            nc.sync.dma_start(out=ore_d[:, :, :], in_=out_re)
            tsum = ppool.tile([TP, 1, KK, FB], bf16, tag="t0", name="tsum")
            nc.vector.tensor_add(tsum[:, 0], A, Bc)
            out_im = opool.tile([TP, KK, FB], bf16, tag="oim")
            nc.vector.tensor_sub(out_im, Cc, tsum[:, 0])       # im = C-(A+B)
            nc.scalar.dma_start(out=oim_d[:, :, :], in_=out_im)

    nc.finalize()
    return nc


def _get_program():
    if "nc" not in _prog_cache:
        _prog_cache["nc"] = _build_program()
    return _prog_cache["nc"]


def _host_prep(m, x):
    import ml_dtypes

    bf = ml_dtypes.bfloat16
    in_maps = []
    for b in range(B):
        mr = m[b].reshape(3, 9, T, F)
        hre = mr[0] - 0.5 * (mr[1] + mr[2])
        him = SQ3_2 * (mr[1] - mr[2])
        # [2ch, 9, T, F] -> [TP, 9(tap order), 2, KK, FB]; t = kk*TP + p
        mk = np.zeros((2, 9, KK, TP, FB), np.float32)
        st = np.stack([hre, him])[:, list(TAP_ORDER)]  # (2, 9, T, F)
        mk[:, :, :, :, :F] = st.reshape(2, 9, KK, TP, F)
        mk = np.ascontiguousarray(mk.transpose(3, 1, 0, 2, 4)).astype(bf)

        xb = x[b]  # (F, T, 2)
        xrp = np.zeros((T + 2, XE + 2), np.float32)
        xip = np.zeros((T + 2, XE + 2), np.float32)
        xrp[2:, 1:F + 1] = xb[:, :, 0].T
        xip[2:, 1:F + 1] = xb[:, :, 1].T
        sp = xrp + xip
        planes = [xrp, xip]
        xe = np.empty((3, TP, 2, KK, XE), np.float32)
        xo = np.empty((3, TP, 2, KK, XO), np.float32)
        se = np.empty((3, TP, KK, XE), np.float32)
        so = np.empty((3, TP, KK, XO), np.float32)
        for rep in range(3):
            for kk in range(KK):
                # rows t = kk*TP + p, padded row index t + rep
                r0 = kk * TP + rep
                for pl in range(2):
                    xe[rep, :, pl, kk, :] = planes[pl][r0:r0 + TP, 0:XE]
                    xo[rep, :, pl, kk, :] = planes[pl][r0:r0 + TP, 1:1 + XO]
                se[rep, :, kk, :] = sp[r0:r0 + TP, 0:XE]
                so[rep, :, kk, :] = sp[r0:r0 + TP, 1:1 + XO]
        in_maps.append({"mk": mk, "xe": xe.astype(bf), "xo": xo.astype(bf),
                        "se": se.astype(bf), "so": so.astype(bf)})
    return in_maps


def _assemble(results):
    out = np.empty((B, F, T, 2), np.float32)
    for b in range(B):
        for ci, name in enumerate(("ore", "oim")):
            arr = results[b][name].astype(np.float32)  # [TP, KK, FB]
            a = arr[:, :, :F].transpose(1, 0, 2).reshape(T, F)  # t = kk*TP+p
            out[b, :, :, ci] = a.T
    return out


def kernel(m, x, _trace=False):
    from concourse.bass_utils import run_bass_kernel_spmd

    nc = _get_program()
    in_maps = _host_prep(np.asarray(m), np.asarray(x))
    res = run_bass_kernel_spmd(nc, in_maps, list(range(B)), trace=_trace)
    out = _assemble(res.results)
    if _trace:
        return out, res
    return out


# revision 24
# speedup vs baseline: 1.1371x; 1.0024x over previous
"""Trainium2 Bass kernel for nn_CCM: per-pixel complex 3x3 mask stencil.

Computation (per batch b):
  H_c = m[c] + v1*m[9+c] + v2*m[18+c],  v1/v2 = -1/2 +- i*sqrt(3)/2, c in 0..8
  out(t,f) = sum_c H_c(t,f) * xpad(t + c//3, f + c%3)   (complex)
with xpad zero-padded by 2 rows at the top (causal time) and 1 col each side.

Sharding: pure data-parallel over B=8 across the 8 NeuronCores.

v4.5 design (informed by per-run traces):
  - Host precomputes mask channels hre_c, him_c (fp32 -> bf16); the
    Karatsuba sum channel hsum=hre+him is derived on-chip (1 DVE add
    per tap).  Host also ships s=xr+xi planes.
  - One stacked DVE product op per tap over the (k1|k2|k3) plane axis;
    in-place balanced-tree accumulation; bf16 2x mode throughout.
    GpSimd does no elementwise work (SBUF-port contention).
  - DMA split by measured path capability:
      SWDGE (gpsimd, 16 engines, ~14-18GB/s each): 9 single-tap mask
        DMAs (ring of 4 for lookahead) + 6 s-plane chunks.
      HWDGE (sync/scalar, 5 shared engines, ~26GB/s each): x planes
        (xr, xi) both parities, plus the two output stores.
    Every transfer is >=8KB contiguous per partition.
  - x planes replicated for the 3 row shifts and duplicated at two
    byte parities so all product slices stay 4B-aligned (2x mode).
  - Error ~1.3e-2 scale-relative (budget 2e-2).
"""

import sys

import numpy as np

sys.path.insert(0, "/opt/trn_rl_repo")

B, T, F = 8, 1000, 257
TP = 125          # partitions; time row t = kk*TP + p
KK = 8            # time chunks
FB = 258          # padded op width (even element count for bf16 2x mode)
XE = 260          # even-parity x row width (covers col shifts 0 and 2)
XO = 258          # odd-parity x row width (col shift 1, pre-shifted)
SQ3_2 = float(np.sqrt(3.0) / 2.0)

# tap compute order; masks are stored in this order tap-major
TAP_ORDER = (0, 2, 1, 3, 5, 4, 6, 8, 7)

_prog_cache = {}


def _build_program():
    import concourse.tile as tile
    from concourse import bacc, mybir

    bf16 = mybir.dt.bfloat16

    nc = bacc.Bacc()
    # (hre, him) per tap, partition-major, tap axis in TAP_ORDER
    mk_d = nc.declare_dram_parameter("mk", [TP, 9, 2, KK, FB], bf16,
                                     isOutput=False)
    # x planes (xr, xi) per (rowshift rep, parity)
    xe_d = nc.declare_dram_parameter("xe", [3, TP, 2, KK, XE], bf16,
                                     isOutput=False)
    xo_d = nc.declare_dram_parameter("xo", [3, TP, 2, KK, XO], bf16,
                                     isOutput=False)
    ore_d = nc.declare_dram_parameter("ore", [TP, KK, FB], bf16,
                                      isOutput=True)
    oim_d = nc.declare_dram_parameter("oim", [TP, KK, FB], bf16,
                                      isOutput=True)

    with tile.TileContext(nc) as tc:
        from contextlib import ExitStack

        with ExitStack() as ctx:
            xpool = ctx.enter_context(tc.tile_pool(name="xpool", bufs=1))
            mpool = ctx.enter_context(tc.tile_pool(name="mpool", bufs=4))
            ppool = ctx.enter_context(tc.tile_pool(name="ppool", bufs=1))
            opool = ctx.enter_context(tc.tile_pool(name="opool", bufs=1))

            xe_t = {}
            xo_t = {}
            mk_t = {}

            # Everything rides ONE SWDGE queue: concurrent HWDGE
            # traffic oversubscribes the 5 shared SDMA engines and
            # delays the completion of EVERY SWDGE DMA (each needs all
            # 16 engines to finish).  HWDGE is used only for the final
            # output stores, after the SWDGE stream has drained.
            def load_x(rep):
                te = xpool.tile([TP, 3, KK, XE], bf16, tag=f"xe{rep}",
                                name=f"xe{rep}")
                nc.gpsimd.dma_start(out=te[:, 0:2], in_=xe_d[rep])
                xe_t[rep] = te
                to = xpool.tile([TP, 3, KK, XO], bf16, tag=f"xo{rep}",
                                name=f"xo{rep}")
                nc.gpsimd.dma_start(out=to[:, 0:2], in_=xo_d[rep])
                xo_t[rep] = to

            def load_mk(oi):
                t = mpool.tile([TP, 3, KK, FB], bf16, tag="mk",
                               name=f"mk{oi}")
                nc.gpsimd.dma_start(out=t[:, 0:2], in_=mk_d[:, oi])
                mk_t[TAP_ORDER[oi]] = t

            load_mk(0)
            load_x(0)
            load_mk(1)
            load_mk(2)
            load_x(1)
            load_mk(3)
            load_mk(4)
            load_x(2)
            load_mk(5)
            load_mk(6)
            load_mk(7)
            load_mk(8)

            # ---- Compute: hsum derivation + one stacked product per
            # tap; balanced pairwise tree with in-place accumulation:
            #   (((0+2)+(1+3)) + ((5+4)+(6+8))) + 7
            def prod_op(c, tag, bufs=1):
                mm, nn = divmod(c, 3)
                if nn == 1:
                    xs = xo_t[mm][:, :, :, 0:FB]
                else:
                    xs = xe_t[mm][:, :, :, nn:nn + FB]
                mk = mk_t[c]
                nc.vector.tensor_add(mk[:, 2], mk[:, 0], mk[:, 1])
                p = ppool.tile([TP, 3, KK, FB], bf16, tag=tag, bufs=bufs,
                               name=f"p{c}")
                nc.vector.tensor_mul(p, mk, xs)
                return p

            def s_add(t):
                nc.vector.tensor_add(t[:, 2], t[:, 0], t[:, 1])

            s_add(xe_t[0])
            acc0 = prod_op(0, "acc0")                 # p0
            f0 = prod_op(2, "feed", bufs=2)
            nc.vector.tensor_add(acc0, acc0, f0)      # 0+2
            s_add(xo_t[0])
            t0 = prod_op(1, "t0")
            s_add(xe_t[1])
            f1 = prod_op(3, "feed", bufs=2)
            nc.vector.tensor_add(t0, t0, f1)          # 1+3
            nc.vector.tensor_add(acc0, acc0, t0)      # (0+2)+(1+3)
            acc1 = prod_op(5, "acc1")
            s_add(xo_t[1])
            f2 = prod_op(4, "feed", bufs=2)
            nc.vector.tensor_add(acc1, acc1, f2)      # 5+4
            s_add(xe_t[2])
            t1 = prod_op(6, "t1")
            s_add(xo_t[2])
            f3 = prod_op(8, "feed", bufs=2)
            nc.vector.tensor_add(t1, t1, f3)          # 6+8
            nc.vector.tensor_add(acc1, acc1, t1)      # (5+4)+(6+8)
            nc.vector.tensor_add(acc0, acc0, acc1)    # left + right
            f4 = prod_op(7, "feed", bufs=2)
            nc.vector.tensor_add(acc0, acc0, f4)      # ... + 7
            A = acc0[:, 0]
            Bc = acc0[:, 1]
            Cc = acc0[:, 2]

            # merges; store re and im on separate HWDGE queues so the
            # stores overlap the remaining compute
            out_re = opool.tile([TP, KK, FB], bf16, tag="ore")
            nc.vector.tensor_sub(out_re, A, Bc)                # re = A - B
            nc.sync.dma_start(out=ore_d[:, :, :], in_=out_re)
            tsum = ppool.tile([TP, 1, KK, FB], bf16, tag="t0", name="tsum")
            nc.vector.tensor_add(tsum[:, 0], A, Bc)
            out_im = opool.tile([TP, KK, FB], bf16, tag="oim")
            nc.vector.tensor_sub(out_im, Cc, tsum[:, 0])       # im = C-(A+B)
            nc.scalar.dma_start(out=oim_d[:, :, :], in_=out_im)

    nc.finalize()
    return nc


def _get_program():
    if "nc" not in _prog_cache:
        _prog_cache["nc"] = _build_program()
    return _prog_cache["nc"]


def _host_prep(m, x):
    import ml_dtypes

    bf = ml_dtypes.bfloat16
    in_maps = []
    for b in range(B):
        mr = m[b].reshape(3, 9, T, F)
        hre = mr[0] - 0.5 * (mr[1] + mr[2])
        him = SQ3_2 * (mr[1] - mr[2])
        # [2ch, 9, T, F] -> [TP, 9(tap order), 2, KK, FB]; t = kk*TP + p
        mk = np.zeros((2, 9, KK, TP, FB), np.float32)
        st = np.stack([hre, him])[:, list(TAP_ORDER)]  # (2, 9, T, F)
        mk[:, :, :, :, :F] = st.reshape(2, 9, KK, TP, F)
        mk = np.ascontiguousarray(mk.transpose(3, 1, 0, 2, 4)).astype(bf)

        xb = x[b]  # (F, T, 2)
        xrp = np.zeros((T + 2, XE + 2), np.float32)
        xip = np.zeros((T + 2, XE + 2), np.float32)
        xrp[2:, 1:F + 1] = xb[:, :, 0].T
        xip[2:, 1:F + 1] = xb[:, :, 1].T
        planes = [xrp, xip]
        xe = np.empty((3, TP, 2, KK, XE), np.float32)
        xo = np.empty((3, TP, 2, KK, XO), np.float32)
        for rep in range(3):
            for kk in range(KK):
                # rows t = kk*TP + p, padded row index t + rep
                r0 = kk * TP + rep
                for pl in range(2):
                    xe[rep, :, pl, kk, :] = planes[pl][r0:r0 + TP, 0:XE]
                    xo[rep, :, pl, kk, :] = planes[pl][r0:r0 + TP, 1:1 + XO]
        in_maps.append({"mk": mk, "xe": xe.astype(bf), "xo": xo.astype(bf)})
    return in_maps


def _assemble(results):
    out = np.empty((B, F, T, 2), np.float32)
    for b in range(B):
        for ci, name in enumerate(("ore", "oim")):
            arr = results[b][name].astype(np.float32)  # [TP, KK, FB]
            a = arr[:, :, :F].transpose(1, 0, 2).reshape(T, F)  # t = kk*TP+p
            out[b, :, :, ci] = a.T
    return out


def kernel(m, x, _trace=False):
    from concourse.bass_utils import run_bass_kernel_spmd

    nc = _get_program()
    in_maps = _host_prep(np.asarray(m), np.asarray(x))
    res = run_bass_kernel_spmd(nc, in_maps, list(range(B)), trace=_trace)
    out = _assemble(res.results)
    if _trace:
        return out, res
    return out


# revision 25
# speedup vs baseline: 1.2349x; 1.0860x over previous
"""Trainium2 Bass kernel for nn_CCM: per-pixel complex 3x3 mask stencil.

Computation (per batch b):
  H_c = m[c] + v1*m[9+c] + v2*m[18+c],  v1/v2 = -1/2 +- i*sqrt(3)/2, c in 0..8
  out(t,f) = sum_c H_c(t,f) * xpad(t + c//3, f + c%3)   (complex)
with xpad zero-padded by 2 rows at the top (causal time) and 1 col each side.

Sharding: pure data-parallel over B=8 across the 8 NeuronCores.

v4.5 design (informed by per-run traces):
  - Host precomputes mask channels hre_c, him_c (fp32 -> bf16); the
    Karatsuba sum channel hsum=hre+him is derived on-chip (1 DVE add
    per tap).  Host also ships s=xr+xi planes.
  - One stacked DVE product op per tap over the (k1|k2|k3) plane axis;
    in-place balanced-tree accumulation; bf16 2x mode throughout.
    GpSimd does no elementwise work (SBUF-port contention).
  - DMA split by measured path capability:
      SWDGE (gpsimd, 16 engines, ~14-18GB/s each): 9 single-tap mask
        DMAs (ring of 4 for lookahead) + 6 s-plane chunks.
      HWDGE (sync/scalar, 5 shared engines, ~26GB/s each): x planes
        (xr, xi) both parities, plus the two output stores.
    Every transfer is >=8KB contiguous per partition.
  - x planes replicated for the 3 row shifts and duplicated at two
    byte parities so all product slices stay 4B-aligned (2x mode).
  - Error ~1.3e-2 scale-relative (budget 2e-2).
"""

import sys

import numpy as np

sys.path.insert(0, "/opt/trn_rl_repo")

B, T, F = 8, 1000, 257
TP = 125          # partitions; time row t = kk*TP + p
KK = 8            # time chunks
FB = 258          # padded op width (even element count for bf16 2x mode)
XE = 260          # even-parity x row width (covers col shifts 0 and 2)
XO = 258          # odd-parity x row width (col shift 1, pre-shifted)
SQ3_2 = float(np.sqrt(3.0) / 2.0)

# tap compute order; masks are stored in this order tap-major
TAP_ORDER = (0, 2, 1, 3, 5, 4, 6, 8, 7)

_prog_cache = {}


def _build_program():
    import concourse.tile as tile
    from concourse import bacc, mybir

    bf16 = mybir.dt.bfloat16

    nc = bacc.Bacc()
    # (hre, him) per tap, partition-major, tap axis in TAP_ORDER
    mk_d = nc.declare_dram_parameter("mk", [TP, 9, 2, KK, FB], bf16,
                                     isOutput=False)
    # x planes (xr, xi) per (rowshift rep, parity)
    xe_d = nc.declare_dram_parameter("xe", [3, TP, 2, KK, XE], bf16,
                                     isOutput=False)
    xo_d = nc.declare_dram_parameter("xo", [3, TP, 2, KK, XO], bf16,
                                     isOutput=False)
    ore_d = nc.declare_dram_parameter("ore", [TP, KK, FB], bf16,
                                      isOutput=True)
    oim_d = nc.declare_dram_parameter("oim", [TP, KK, FB], bf16,
                                      isOutput=True)

    with tile.TileContext(nc) as tc:
        from contextlib import ExitStack

        with ExitStack() as ctx:
            xpool = ctx.enter_context(tc.tile_pool(name="xpool", bufs=1))
            mpool = ctx.enter_context(tc.tile_pool(name="mpool", bufs=5))
            ppool = ctx.enter_context(tc.tile_pool(name="ppool", bufs=1))
            opool = ctx.enter_context(tc.tile_pool(name="opool", bufs=1))

            xe_t = {}
            xo_t = {}
            mk_t = {}

            # Everything rides ONE SWDGE queue: concurrent HWDGE
            # traffic oversubscribes the 5 shared SDMA engines and
            # delays the completion of EVERY SWDGE DMA (each needs all
            # 16 engines to finish).  HWDGE is used only for the final
            # output stores, after the SWDGE stream has drained.
            def load_x(rep):
                te = xpool.tile([TP, 3, KK, XE], bf16, tag=f"xe{rep}",
                                name=f"xe{rep}")
                nc.gpsimd.dma_start(out=te[:, 0:2], in_=xe_d[rep])
                xe_t[rep] = te
                to = xpool.tile([TP, 3, KK, XO], bf16, tag=f"xo{rep}",
                                name=f"xo{rep}")
                nc.gpsimd.dma_start(out=to[:, 0:2], in_=xo_d[rep])
                xo_t[rep] = to

            def load_mk(oi):
                t = mpool.tile([TP, 3, KK, FB], bf16, tag="mk",
                               name=f"mk{oi}")
                nc.gpsimd.dma_start(out=t[:, 0:2], in_=mk_d[:, oi])
                mk_t[TAP_ORDER[oi]] = t

            load_mk(0)
            load_x(0)
            load_mk(1)
            load_mk(2)
            load_x(1)
            load_mk(3)
            load_mk(4)
            load_x(2)
            load_mk(5)
            load_mk(6)
            load_mk(7)
            load_mk(8)

            # ---- Compute: hsum derivation + one stacked product per
            # tap; balanced pairwise tree with in-place accumulation:
            #   (((0+2)+(1+3)) + ((5+4)+(6+8))) + 7
            def prod_op(c, tag, bufs=1):
                mm, nn = divmod(c, 3)
                if nn == 1:
                    xs = xo_t[mm][:, :, :, 0:FB]
                else:
                    xs = xe_t[mm][:, :, :, nn:nn + FB]
                mk = mk_t[c]
                nc.vector.tensor_add(mk[:, 2], mk[:, 0], mk[:, 1])
                p = ppool.tile([TP, 3, KK, FB], bf16, tag=tag, bufs=bufs,
                               name=f"p{c}")
                nc.vector.tensor_mul(p, mk, xs)
                return p

            def s_add(t):
                nc.vector.tensor_add(t[:, 2], t[:, 0], t[:, 1])

            s_add(xe_t[0])
            acc0 = prod_op(0, "acc0")                 # p0
            f0 = prod_op(2, "feed", bufs=2)
            nc.vector.tensor_add(acc0, acc0, f0)      # 0+2
            s_add(xo_t[0])
            t0 = prod_op(1, "tp")
            s_add(xe_t[1])
            f1 = prod_op(3, "feed", bufs=2)
            nc.vector.tensor_add(t0, t0, f1)          # 1+3
            nc.vector.tensor_add(acc0, acc0, t0)      # (0+2)+(1+3)
            acc1 = prod_op(5, "acc1")
            s_add(xo_t[1])
            f2 = prod_op(4, "feed", bufs=2)
            nc.vector.tensor_add(acc1, acc1, f2)      # 5+4
            s_add(xe_t[2])
            t1 = prod_op(6, "tp")
            s_add(xo_t[2])
            f3 = prod_op(8, "feed", bufs=2)
            nc.vector.tensor_add(t1, t1, f3)          # 6+8
            nc.vector.tensor_add(acc1, acc1, t1)      # (5+4)+(6+8)
            nc.vector.tensor_add(acc0, acc0, acc1)    # left + right
            f4 = prod_op(7, "feed", bufs=2)
            nc.vector.tensor_add(acc0, acc0, f4)      # ... + 7
            A = acc0[:, 0]
            Bc = acc0[:, 1]
            Cc = acc0[:, 2]

            # merges + stores split by kk halves on the two HWDGE
            # queues so stores overlap the remaining merge compute
            H = KK // 2
            out_re = opool.tile([TP, KK, FB], bf16, tag="ore")
            out_im = opool.tile([TP, KK, FB], bf16, tag="oim")
            tsum = ppool.tile([TP, 1, KK, FB], bf16, tag="tp", name="tsum")
            halves = ((slice(0, H), nc.sync, 0), (slice(H, KK), nc.scalar, H))
            for sl, eng, kk0 in halves:
                nc.vector.tensor_sub(out_re[:, sl], A[:, sl], Bc[:, sl])
                eng.dma_start(out=ore_d[:, sl, :], in_=out_re[:, sl])
            for sl, eng, kk0 in halves:
                nc.vector.tensor_add(tsum[:, 0, sl], A[:, sl], Bc[:, sl])
                nc.vector.tensor_sub(out_im[:, sl], Cc[:, sl], tsum[:, 0, sl])
                eng.dma_start(out=oim_d[:, sl, :], in_=out_im[:, sl])

    nc.finalize()
    return nc


def _get_program():
    if "nc" not in _prog_cache:
        _prog_cache["nc"] = _build_program()
    return _prog_cache["nc"]


def _host_prep(m, x):
    import ml_dtypes

    bf = ml_dtypes.bfloat16
    in_maps = []
    for b in range(B):
        mr = m[b].reshape(3, 9, T, F)
        hre = mr[0] - 0.5 * (mr[1] + mr[2])
        him = SQ3_2 * (mr[1] - mr[2])
        # [2ch, 9, T, F] -> [TP, 9(tap order), 2, KK, FB]; t = kk*TP + p
        mk = np.zeros((2, 9, KK, TP, FB), np.float32)
        st = np.stack([hre, him])[:, list(TAP_ORDER)]  # (2, 9, T, F)
        mk[:, :, :, :, :F] = st.reshape(2, 9, KK, TP, F)
        mk = np.ascontiguousarray(mk.transpose(3, 1, 0, 2, 4)).astype(bf)

        xb = x[b]  # (F, T, 2)
        xrp = np.zeros((T + 2, XE + 2), np.float32)
        xip = np.zeros((T + 2, XE + 2), np.float32)
        xrp[2:, 1:F + 1] = xb[:, :, 0].T
        xip[2:, 1:F + 1] = xb[:, :, 1].T
        planes = [xrp, xip]
        xe = np.empty((3, TP, 2, KK, XE), np.float32)
        xo = np.empty((3, TP, 2, KK, XO), np.float32)
        for rep in range(3):
            for kk in range(KK):
                # rows t = kk*TP + p, padded row index t + rep
                r0 = kk * TP + rep
                for pl in range(2):
                    xe[rep, :, pl, kk, :] = planes[pl][r0:r0 + TP, 0:XE]
                    xo[rep, :, pl, kk, :] = planes[pl][r0:r0 + TP, 1:1 + XO]
        in_maps.append({"mk": mk, "xe": xe.astype(bf), "xo": xo.astype(bf)})
    return in_maps


def _assemble(results):
    out = np.empty((B, F, T, 2), np.float32)
    for b in range(B):
        for ci, name in enumerate(("ore", "oim")):
            arr = results[b][name].astype(np.float32)  # [TP, KK, FB]
            a = arr[:, :, :F].transpose(1, 0, 2).reshape(T, F)  # t = kk*TP+p
            out[b, :, :, ci] = a.T
    return out


def kernel(m, x, _trace=False):
    from concourse.bass_utils import run_bass_kernel_spmd

    nc = _get_program()
    in_maps = _host_prep(np.asarray(m), np.asarray(x))
    res = run_bass_kernel_spmd(nc, in_maps, list(range(B)), trace=_trace)
    out = _assemble(res.results)
    if _trace:
        return out, res
    return out
